# revision 1
# baseline (speedup 1.0000x reference)
"""AudioEncoder Trainium2 kernel.

Computes: conv1d(1->64, k=5, stride=2, pad=2) + bias -> ReLU -> per-timestep
linear (64->64) + bias, over audio [4, 480000] f32 -> out [4, 240000, 64] f32.

Strategy (pure data parallel over 8 cores):
  - Each core handles one half of one batch row: S = 120000 output positions.
  - Host pre-pads/casts audio to fp16 and de-interleaves it into even/odd
    streams xe[i] = xp[2i], xo[i] = xp[2i+1] (xp[t] = x[t-2] zero-padded), so
    the on-chip im2col rows are contiguous DMA reads:
      row0 = xe[j]   (tap 0)   row3 = xo[j]   (tap 1)
      row1 = xe[1+j] (tap 2)   row4 = xo[1+j] (tap 3)
      row2 = xe[2+j] (tap 4)
    (conv weights are host-reordered to [k=0,2,4,1,3] to match).
  - Conv: K=5 fp16 matmul; the moving operand uses a permuted 3D AP so that
    within each 512-position half, PSUM column c = t*128 + r holds position
    j0 + 4r + t.  Two col-group-packed matmuls fill PSUM [128, 512] with two
    halves (partitions 0-63 / 64-127).
  - ACT applies conv bias + ReLU, evacuating PSUM -> SBUF fp16 feats.
  - Linear: K=64 fp16 matmuls, feats tiles stationary, lin_w.T moving.  The
    A (feats rows 0-63) and B (rows 64-127) matmuls write SEPARATE PSUM banks:
    row-group-tiled matmuls writing the same partitions of one bank
    concurrently is a hardware fault (per-partition PSUM write-port conflict).
  - DVE adds the (pre-broadcast) linear bias while evacuating to SBUF.
  - Thanks to the position permutation each SBUF partition holds 4+4
    consecutive output rows, so the store DMA moves 1 KiB-contiguous runs.
"""

import numpy as np

import concourse.bacc as bacc
import concourse.bass as bass
import concourse.mybir as mybir
import concourse.tile as tile
from concourse.bass_utils import run_bass_kernel_spmd

B = 4
T = 480000
S_FULL = 240000  # conv output positions per batch row
N_CORES = 8
S_CORE = S_FULL * B // N_CORES  # 120000 positions per core
CHUNK = 1024  # output positions per inner chunk (two 512 halves)
SUPER = 8192  # output positions covered per im2col load
E = 64  # conv out channels
P = 64  # linear out features
KS = 5

f16 = mybir.dt.float16
f32 = mybir.dt.float32


def emit(nc: bass.Bass, S: int = S_CORE) -> None:
    """Emit the per-core Tile kernel for S output positions."""
    from contextlib import ExitStack

    xe_d = nc.declare_dram_parameter("xe", [S + 2], f16, isOutput=False)
    xo_d = nc.declare_dram_parameter("xo", [S + 2], f16, isOutput=False)
    wc_d = nc.declare_dram_parameter("wc", [KS, E], f16, isOutput=False)
    cb_d = nc.declare_dram_parameter("cb", [128, 1], f32, isOutput=False)
    w2_d = nc.declare_dram_parameter("w2", [128, P], f16, isOutput=False)
    b2_d = nc.declare_dram_parameter("b2", [128, 8 * P], f32, isOutput=False)
    out_d = nc.declare_dram_parameter("out", [S, P], f32, isOutput=True)

    RELU = mybir.ActivationFunctionType.Relu
    HALF = CHUNK // 2

    with tile.TileContext(nc) as tc, ExitStack() as ctx:
        consts = ctx.enter_context(tc.tile_pool(name="consts", bufs=1))
        imp = ctx.enter_context(tc.tile_pool(name="im", bufs=2))
        fpool = ctx.enter_context(tc.tile_pool(name="feats", bufs=3))
        opool = ctx.enter_context(tc.tile_pool(name="outs", bufs=3))
        pc = ctx.enter_context(tc.tile_pool(name="psc", bufs=2, space="PSUM"))
        plA = ctx.enter_context(tc.tile_pool(name="pslA", bufs=2, space="PSUM"))
        plB = ctx.enter_context(tc.tile_pool(name="pslB", bufs=2, space="PSUM"))

        wc_sb = consts.tile([KS, E], f16)
        nc.sync.dma_start(out=wc_sb[:, :], in_=wc_d[:, :])
        cb_sb = consts.tile([128, 1], f32)
        nc.sync.dma_start(out=cb_sb[:, :], in_=cb_d[:, :])
        w2_sb = consts.tile([128, P], f16)
        nc.sync.dma_start(out=w2_sb[:, :], in_=w2_d[:, :])
        b2_sb = consts.tile([128, 8 * P], f32)
        nc.sync.dma_start(out=b2_sb[:, :], in_=b2_d[:, :])

        n_super = (S + SUPER - 1) // SUPER
        for sc in range(n_super):
            sbase = sc * SUPER
            scount = min(SUPER, S - sbase)
            im = imp.tile([KS, SUPER], f16)
            # rows 0-2: xe shifted 0/1/2; rows 3-4: xo shifted 0/1 — both
            # contiguous in DRAM (overlapping row reads are fine).
            nc.sync.dma_start(
                out=im[0:3, 0:scount],
                in_=bass.AP(tensor=xe_d, offset=sbase, ap=[[1, 3], [1, scount]]),
            )
            nc.sync.dma_start(
                out=im[3:5, 0:scount],
                in_=bass.AP(tensor=xo_d, offset=sbase, ap=[[1, 2], [1, scount]]),
            )

            cbase = 0
            while cbase < scount:
                cn = min(CHUNK, scount - cbase)
                assert cn % 2 == 0
                nA = cn // 2
                j0 = cbase
                p0g = sbase + cbase  # global first position of this chunk
                full = nA == HALF

                # conv: two halves -> PSUM partitions 0-63 / 64-127.
                psc = pc.tile([128, HALF], f32)
                if full:
                    # permuted moving operand: psum col t*128 + r holds
                    # position j0 + 4r + t
                    rhsA = im[:, j0 : j0 + nA].rearrange("k (r t) -> k t r", t=4)
                    rhsB = im[:, j0 + nA : j0 + 2 * nA].rearrange(
                        "k (r t) -> k t r", t=4
                    )
                else:
                    rhsA = im[:, j0 : j0 + nA]
                    rhsB = im[:, j0 + nA : j0 + 2 * nA]
                nc.tensor.matmul(
                    out=psc[0:E, 0:nA], lhsT=wc_sb[:, :], rhs=rhsA,
                    start=True, stop=True,
                )
                nc.tensor.matmul(
                    out=psc[E : 2 * E, 0:nA], lhsT=wc_sb[:, :], rhs=rhsB,
                    start=True, stop=True,
                )

                feats = fpool.tile([128, HALF], f16)
                nc.scalar.activation(
                    out=feats[:, 0:nA], in_=psc[:, 0:nA], func=RELU,
                    bias=cb_sb[:, 0:1], scale=1.0,
                )

                # linear: position tiles of <=128 as stationary operands.
                m_tiles = [
                    (i * 128, min(128, nA - i * 128)) for i in range((nA + 127) // 128)
                ]
                mlen0 = m_tiles[0][1]
                assert all(ml == mlen0 for _, ml in m_tiles)
                nb = len(m_tiles)
                psA = plA.tile([128, HALF // 2], f32)
                psB = plB.tile([128, HALF // 2], f32)
                for bi, (mo, ml) in enumerate(m_tiles):
                    nc.tensor.matmul(
                        out=psA[0:ml, P * bi : P * bi + P],
                        lhsT=feats[0:E, mo : mo + ml],
                        rhs=w2_sb[0:E, :], start=True, stop=True,
                    )
                    nc.tensor.matmul(
                        out=psB[0:ml, P * bi : P * bi + P],
                        lhsT=feats[E : 2 * E, mo : mo + ml],
                        rhs=w2_sb[E : 2 * E, :], start=True, stop=True,
                    )

                ncols = nb * P
                outt = opool.tile([128, HALF], f32)
                nc.vector.tensor_add(
                    outt[0:mlen0, 0:ncols],
                    psA[0:mlen0, 0:ncols],
                    b2_sb[0:mlen0, 0:ncols],
                )
                nc.vector.tensor_add(
                    outt[0:mlen0, ncols : 2 * ncols],
                    psB[0:mlen0, 0:ncols],
                    b2_sb[0:mlen0, 0:ncols],
                )

                if full:
                    # s = p0g + h*512 + 4r + q ; sbuf col = h*256 + q*64 + p
                    dview = out_d[p0g : p0g + cn, :].rearrange(
                        "(h r q) p -> r h q p", h=2, q=4
                    )
                    sview = outt[:, :].rearrange("r (h q p) -> r h q p", h=2, q=4)
                else:
                    # s = p0g + h*nA + r ; sbuf col = h*64 + p
                    dview = out_d[p0g : p0g + cn, :].rearrange(
                        "(h r) p -> r h p", h=2
                    )
                    sview = outt[0:mlen0, 0 : 2 * ncols].rearrange(
                        "r (h p) -> r h p", h=2
                    )
                nc.sync.dma_start(out=dview, in_=sview)

                cbase += cn


def prep_shared(conv_w, conv_b, lin_w, lin_b):
    """Host-side prep of the (tiny, replicated) parameter tensors."""
    conv_w = np.asarray(conv_w, dtype=np.float32)
    conv_b = np.asarray(conv_b, dtype=np.float32)
    lin_w = np.asarray(lin_w, dtype=np.float32)
    lin_b = np.asarray(lin_b, dtype=np.float32)

    wk = conv_w[:, 0, :]  # [64, 5]
    wc = np.ascontiguousarray(wk[:, [0, 2, 4, 1, 3]].T).astype(np.float16)  # [5, 64]
    cb = np.ascontiguousarray(
        np.concatenate([conv_b, conv_b]).astype(np.float32)[:, None]
    )  # [128, 1]
    w2 = lin_w.T.astype(np.float16)  # [64e, 64p]
    w2s = np.ascontiguousarray(np.concatenate([w2, w2], axis=0))  # [128, 64]
    b2 = np.ascontiguousarray(
        np.tile(lin_b.astype(np.float32)[None, :], (128, 8))
    )  # [128, 512]
    return wc, cb, w2s, b2


def prep_inputs(audio_waveform, conv_w, conv_b, lin_w, lin_b):
    """Host-side shard + dtype/layout prep. Returns in_maps for the 8 cores."""
    x = np.asarray(audio_waveform, dtype=np.float32)
    assert x.shape == (B, T)
    xp = np.zeros((B, 2 * S_FULL + 4), dtype=np.float16)
    xp[:, 2 : 2 + T] = x.astype(np.float16)
    xe = xp[:, 0::2]  # [B, S_FULL + 2]
    xo = xp[:, 1::2]  # [B, S_FULL + 2]

    wc, cb, w2s, b2 = prep_shared(conv_w, conv_b, lin_w, lin_b)

    in_maps = []
    for c in range(N_CORES):
        b_i, h = divmod(c, 2)
        s0 = h * S_CORE
        in_maps.append(
            dict(
                xe=np.ascontiguousarray(xe[b_i, s0 : s0 + S_CORE + 2]),
                xo=np.ascontiguousarray(xo[b_i, s0 : s0 + S_CORE + 2]),
                wc=wc, cb=cb, w2=w2s, b2=b2,
            )
        )
    return in_maps


_NC_CACHE = None


def get_nc() -> bass.Bass:
    global _NC_CACHE
    if _NC_CACHE is None:
        nc = bacc.Bacc()
        emit(nc)
        # Legalizes TRN2 sync constraints (splits multi-wait instructions),
        # allocates registers, etc. Required before walrus codegen.
        nc.compile()
        _NC_CACHE = nc
    return _NC_CACHE


def run(inputs: dict, trace: bool = False):
    """Run on the 8 cores; returns (full_output, BassKernelResults)."""
    in_maps = prep_inputs(**inputs)
    nc = get_nc()
    res = run_bass_kernel_spmd(nc, in_maps, list(range(N_CORES)), trace=trace)
    out = np.empty((B, S_FULL, P), dtype=np.float32)
    for c in range(N_CORES):
        b_i, h = divmod(c, 2)
        out[b_i, h * S_CORE : (h + 1) * S_CORE, :] = res.results[c]["out"]
    return out, res


def kernel(**inputs) -> np.ndarray:
    out, _ = run(inputs)
    return out



# revision 6
# speedup vs baseline: 1.3318x; 1.3318x over previous
"""AudioEncoder Trainium2 kernel, v2 (column-major conv scheme).

Computes: conv1d(1->64, k=5, stride=2, pad=2) + bias -> ReLU -> per-timestep
linear (64->64) + bias, over audio [4, 480000] f32 -> out [4, 240000, 64] f32.

Layout strategy (8 cores, pure data parallel; core = half of one batch row,
S = 120000 output positions):

  Host builds xcore [68, 3750] fp16 per core, xcore[p, c] = xpad[64c + p]
  (xpad = zero-padded audio, stride-2 conv => 64 input samples per 32 output
  positions).  Column c covers output positions 32c..32c+31; position
  i within a column needs taps xpad[64c + 2i .. 2i+4].

  Conv as matmul with 16 constant stationaries B_I [68, 128] fp16:
  B_I[p, gpar*64+ch] = wk[ch, p - 2*(I + 16*gpar)].  MM_I: psum[128, ncols]
  = B_I.T @ xcore[:, c0:c0+ncols] gives channels of positions (I, I+16) of
  every column -- plain contiguous moving operand, position permutation is
  free.  ACT applies conv bias + ReLU -> feats_j fp16 (j = I//2, two I per
  [128,1024] psum tile).

  Linear: per col-block of ml<=128 columns, 32 matmuls (g = I + 16*gpar):
  lhsT = feats_j[gpar*64:+64, u*512+m0:+ml] (stationary), rhs = w2 dup
  [gpar*64:+64, 0:64] -> psl[r, g*64+p] = out(pos 32*(c0+m0+r)+g, feat p).
  The gpar=0 / gpar=1 halves write DIFFERENT psum banks: the PE overlaps
  row-group-disjoint matmuls, and two concurrent matmuls writing the same
  partitions of one bank is a hardware fault (write-port conflict).  Within
  a bank all matmuls share one row-group, so they serialize safely.
  DVE adds (pre-broadcast) lin bias, casting f32 psum -> fp16 out tile.

  Store: outt[r, 0:2048] is exactly out[pos0+32r .. pos0+32r+31, 0:64] --
  one fully contiguous 4 KiB descriptor per partition, 512 KiB per store.
  Output is fp16 on device (quantization ~2.4e-4 << 2e-2 tol); host casts
  back to f32.

  PSUM: conv pool 2 bufs x 2 banks + linear pool 2 bufs x 2 banks = 8 banks.
"""

import numpy as np

import concourse.bacc as bacc
import concourse.bass as bass
import concourse.mybir as mybir
import concourse.tile as tile
from concourse.bass_utils import run_bass_kernel_spmd

B = 4
T = 480000
S_FULL = 240000
N_CORES = 8
S_CORE = S_FULL * B // N_CORES  # 120000
NC = S_CORE // 32  # 3750 columns per core
KP = 68  # xcore partition count (max tap index 2*31+4 = 66)
E = 64
P = 64

f16 = mybir.dt.float16
f32 = mybir.dt.float32

SUPERS = [(j * 512, 512) for j in range(NC // 512)] + (
    [((NC // 512) * 512, NC % 512)] if NC % 512 else []
)


def col_blocks(ncols):
    out = []
    m0 = 0
    while m0 < ncols:
        out.append((m0, min(128, ncols - m0)))
        m0 += 128
    return out


def emit(nc: bass.Bass) -> None:
    from contextlib import ExitStack

    xp_d = nc.declare_dram_parameter("xp", [KP, NC], f16, isOutput=False)
    bi_d = nc.declare_dram_parameter("bi", [KP, 16 * 128], f16, isOutput=False)
    w2_d = nc.declare_dram_parameter("w2", [128, P], f16, isOutput=False)
    cb_d = nc.declare_dram_parameter("cb", [128, 1], f32, isOutput=False)
    b2_d = nc.declare_dram_parameter("b2", [128, 8 * P], f32, isOutput=False)
    out_d = nc.declare_dram_parameter("out", [S_CORE, P], f16, isOutput=True)

    RELU = mybir.ActivationFunctionType.Relu

    with tile.TileContext(nc) as tc, ExitStack() as ctx:
        consts = ctx.enter_context(tc.tile_pool(name="consts", bufs=1))
        fpool = ctx.enter_context(tc.tile_pool(name="feats", bufs=16))
        opool = ctx.enter_context(tc.tile_pool(name="outs", bufs=4))
        pc = ctx.enter_context(tc.tile_pool(name="psc", bufs=2, space="PSUM"))
        pl = ctx.enter_context(tc.tile_pool(name="psl", bufs=4, space="PSUM"))

        xp_sb = consts.tile([KP, NC], f16)
        nc.sync.dma_start(out=xp_sb[:, :], in_=xp_d[:, :])
        bi_sb = consts.tile([KP, 16 * 128], f16)
        nc.sync.dma_start(out=bi_sb[:, :], in_=bi_d[:, :])
        w2_sb = consts.tile([128, P], f16)
        nc.sync.dma_start(out=w2_sb[:, :], in_=w2_d[:, :])
        cb_sb = consts.tile([128, 1], f32)
        nc.sync.dma_start(out=cb_sb[:, :], in_=cb_d[:, :])
        b2_sb = consts.tile([128, 8 * P], f32)
        nc.sync.dma_start(out=b2_sb[:, :], in_=b2_d[:, :])

        def emit_conv_step(c0, ncols, j, featss):
            """Conv MMs for I = 2j, 2j+1 into one 2-bank psum tile + ACT."""
            psc = pc.tile([128, 1024], f32)
            for u in range(2):
                I = 2 * j + u
                nc.tensor.matmul(
                    out=psc[:, u * 512 : u * 512 + ncols],
                    lhsT=bi_sb[:, I * 128 : (I + 1) * 128],
                    rhs=xp_sb[:, c0 : c0 + ncols],
                    start=True,
                    stop=True,
                )
            feats = fpool.tile([128, 1024], f16)
            if ncols == 512:
                nc.scalar.activation(
                    out=feats[:, :], in_=psc[:, :], func=RELU,
                    bias=cb_sb[:, 0:1], scale=1.0,
                )
            else:
                for u in range(2):
                    nc.scalar.activation(
                        out=feats[:, u * 512 : u * 512 + ncols],
                        in_=psc[:, u * 512 : u * 512 + ncols],
                        func=RELU, bias=cb_sb[:, 0:1], scale=1.0,
                    )
            featss.append(feats)

        def emit_linear_half(prev, cb_i, h, outt):
            """16 linear MMs (positions-in-col g in {8h..8h+7} u {16+8h..+7}) ->
            two 1-bank psum tiles (A: feats rows 0-63 only, B: rows 64-127
            only -- same-bank MMs share a PE row-group so they serialize;
            A/B pairs overlap across banks), then DVE bias-add into outt."""
            featss, c0, ncols, mlist, _outts = prev
            m0, ml = mlist[cb_i]
            pslA = pl.tile([128, 512], f32, tag="psl")
            pslB = pl.tile([128, 512], f32, tag="psl")
            for gg in range(8):
                for gpar, psl in ((0, pslA), (1, pslB)):
                    I = 8 * h + gg
                    j, u = I // 2, I % 2
                    nc.tensor.matmul(
                        out=psl[0:ml, gg * 64 : (gg + 1) * 64],
                        lhsT=featss[j][
                            gpar * 64 : (gpar + 1) * 64,
                            u * 512 + m0 : u * 512 + m0 + ml,
                        ],
                        rhs=w2_sb[gpar * 64 : (gpar + 1) * 64, :],
                        start=True,
                        stop=True,
                    )
            nc.vector.tensor_add(
                outt[0:ml, h * 512 : (h + 1) * 512],
                pslA[0:ml, 0:512],
                b2_sb[0:ml, 0:512],
            )
            nc.vector.tensor_add(
                outt[0:ml, 1024 + h * 512 : 1024 + (h + 1) * 512],
                pslB[0:ml, 0:512],
                b2_sb[0:ml, 0:512],
            )

        def emit_linear_store(prev, cb_i, outt):
            featss, c0, ncols, mlist, _outts = prev
            m0, ml = mlist[cb_i]
            pos0 = 32 * (c0 + m0)
            dview = out_d[pos0 : pos0 + 32 * ml, :].rearrange(
                "(r g) p -> r (g p)", g=32
            )
            nc.sync.dma_start(out=dview, in_=outt[0:ml, 0 : 32 * P])

        prev = None
        for c0, ncols in SUPERS:
            featss = []
            for j in range(8):
                emit_conv_step(c0, ncols, j, featss)
                if prev is not None and (j // 2) < len(prev[3]):
                    cb_i, u2 = j // 2, j % 2
                    if u2 == 0:
                        prev_outt = opool.tile([128, 2048], f16, tag="outt")
                        prev[4].append(prev_outt)
                    emit_linear_half(prev, cb_i, u2, prev[4][-1])
                    if u2 == 1:
                        emit_linear_store(prev, cb_i, prev[4][-1])
            prev = (featss, c0, ncols, col_blocks(ncols), [])

        # drain: linear for the last super
        featss, c0, ncols, mlist, outts = prev
        for cb_i in range(len(mlist)):
            outt = opool.tile([128, 2048], f16, tag="outt")
            for u2 in range(2):
                emit_linear_half(prev, cb_i, u2, outt)
            emit_linear_store(prev, cb_i, outt)


def prep_shared(conv_w, conv_b, lin_w, lin_b):
    conv_w = np.asarray(conv_w, dtype=np.float32)
    conv_b = np.asarray(conv_b, dtype=np.float32)
    lin_w = np.asarray(lin_w, dtype=np.float32)
    lin_b = np.asarray(lin_b, dtype=np.float32)

    wk = conv_w[:, 0, :]  # [64, 5]
    bi = np.zeros((KP, 16 * 128), dtype=np.float16)
    for I in range(16):
        for gpar in range(2):
            i = I + 16 * gpar
            for k in range(5):
                p = 2 * i + k
                bi[p, I * 128 + gpar * 64 : I * 128 + (gpar + 1) * 64] = wk[
                    :, k
                ].astype(np.float16)
    w2 = lin_w.T.astype(np.float16)  # [64e, 64p]
    w2s = np.ascontiguousarray(np.concatenate([w2, w2], axis=0))  # [128, 64]
    cb = np.ascontiguousarray(
        np.concatenate([conv_b, conv_b]).astype(np.float32)[:, None]
    )  # [128, 1]
    b2 = np.ascontiguousarray(
        np.tile(lin_b.astype(np.float32)[None, :], (128, 8))
    )  # [128, 1024]
    return bi, w2s, cb, b2


def prep_inputs(audio_waveform, conv_w, conv_b, lin_w, lin_b):
    x = np.asarray(audio_waveform, dtype=np.float32)
    assert x.shape == (B, T)
    bi, w2s, cb, b2 = prep_shared(conv_w, conv_b, lin_w, lin_b)

    in_maps = []
    for c in range(N_CORES):
        b_i, h = divmod(c, 2)
        P0 = h * S_CORE
        xpad = np.zeros(2 * T + 8, dtype=np.float16)
        xpad[2 : 2 + T] = x[b_i].astype(np.float16)
        sw = np.lib.stride_tricks.sliding_window_view(xpad, KP)
        xcore = np.ascontiguousarray(
            sw[2 * P0 : 2 * P0 + 64 * NC : 64].T.astype(np.float16)
        )  # [68, 3750]
        in_maps.append(dict(xp=xcore, bi=bi, w2=w2s, cb=cb, b2=b2))
    return in_maps


_NC_CACHE = None


def get_nc() -> bass.Bass:
    global _NC_CACHE
    if _NC_CACHE is None:
        nc = bacc.Bacc()
        emit(nc)
        nc.compile()
        _NC_CACHE = nc
    return _NC_CACHE


def run(inputs: dict, trace: bool = False):
    in_maps = prep_inputs(**inputs)
    nc = get_nc()
    res = run_bass_kernel_spmd(nc, in_maps, list(range(N_CORES)), trace=trace)
    out = np.empty((B, S_FULL, P), dtype=np.float32)
    for c in range(N_CORES):
        b_i, h = divmod(c, 2)
        out[b_i, h * S_CORE : (h + 1) * S_CORE, :] = res.results[c]["out"].astype(
            np.float32
        )
    return out, res


def kernel(**inputs) -> np.ndarray:
    out, _ = run(inputs)
    return out


# revision 7
# speedup vs baseline: 1.5794x; 1.1859x over previous
"""AudioEncoder Trainium2 kernel, v2 (column-major conv scheme).

Computes: conv1d(1->64, k=5, stride=2, pad=2) + bias -> ReLU -> per-timestep
linear (64->64) + bias, over audio [4, 480000] f32 -> out [4, 240000, 64] f32.

Layout strategy (8 cores, pure data parallel; core = half of one batch row,
S = 120000 output positions):

  Host builds xcore [68, 3750] fp16 per core, xcore[p, c] = xpad[64c + p]
  (xpad = zero-padded audio, stride-2 conv => 64 input samples per 32 output
  positions).  Column c covers output positions 32c..32c+31; position
  i within a column needs taps xpad[64c + 2i .. 2i+4].

  Conv as matmul with 16 constant stationaries B_I [68, 128] fp16:
  B_I[p, gpar*64+ch] = wk[ch, p - 2*(I + 16*gpar)].  MM_I: psum[128, ncols]
  = B_I.T @ xcore[:, c0:c0+ncols] gives channels of positions (I, I+16) of
  every column -- plain contiguous moving operand, position permutation is
  free.  ACT applies conv bias + ReLU -> feats_j fp16 (j = I//2, two I per
  [128,1024] psum tile).

  Linear: per col-block of ml<=128 columns, 32 matmuls (g = I + 16*gpar):
  lhsT = feats_j[gpar*64:+64, u*512+m0:+ml] (stationary), rhs = w2 dup
  [gpar*64:+64, 0:64] -> psl[r, g*64+p] = out(pos 32*(c0+m0+r)+g, feat p).
  The gpar=0 / gpar=1 halves write DIFFERENT psum banks: the PE overlaps
  row-group-disjoint matmuls, and two concurrent matmuls writing the same
  partitions of one bank is a hardware fault (write-port conflict).  Within
  a bank all matmuls share one row-group, so they serialize safely.
  DVE adds (pre-broadcast) lin bias, casting f32 psum -> fp16 out tile.

  Store: outt[r, 0:2048] is exactly out[pos0+32r .. pos0+32r+31, 0:64] --
  one fully contiguous 4 KiB descriptor per partition, 512 KiB per store.
  Output is fp16 on device (quantization ~2.4e-4 << 2e-2 tol); host casts
  back to f32.

  PSUM: conv pool 2 bufs x 2 banks + linear pool 2 bufs x 2 banks = 8 banks.
"""

import numpy as np

import concourse.bacc as bacc
import concourse.bass as bass
import concourse.mybir as mybir
import concourse.tile as tile
from concourse.bass_utils import run_bass_kernel_spmd

B = 4
T = 480000
S_FULL = 240000
N_CORES = 8
S_CORE = S_FULL * B // N_CORES  # 120000
NC = S_CORE // 32  # 3750 columns per core
KP = 128  # xcore partitions (taps use rows 0-66; padded to 128 for DMA port
# coverage on loads and full-row stationaries)
E = 64
P = 64

f16 = mybir.dt.float16
f32 = mybir.dt.float32

SUPERS = [(j * 512, 512) for j in range(NC // 512)] + (
    [((NC // 512) * 512, NC % 512)] if NC % 512 else []
)


def col_blocks(ncols):
    out = []
    m0 = 0
    while m0 < ncols:
        out.append((m0, min(128, ncols - m0)))
        m0 += 128
    return out


def emit(nc: bass.Bass) -> None:
    from contextlib import ExitStack

    xp_d = nc.declare_dram_parameter("xp", [KP, NC], f16, isOutput=False)
    bi_d = nc.declare_dram_parameter("bi", [KP, 16 * 128], f16, isOutput=False)
    w2_d = nc.declare_dram_parameter("w2", [128, P], f16, isOutput=False)
    cb_d = nc.declare_dram_parameter("cb", [128, 1], f32, isOutput=False)
    b2_d = nc.declare_dram_parameter("b2", [128, 8 * P], f32, isOutput=False)
    out_d = nc.declare_dram_parameter("out", [S_CORE, P], f16, isOutput=True)

    RELU = mybir.ActivationFunctionType.Relu

    with tile.TileContext(nc) as tc, ExitStack() as ctx:
        consts = ctx.enter_context(tc.tile_pool(name="consts", bufs=1))
        fpool = ctx.enter_context(tc.tile_pool(name="feats", bufs=16))
        opool = ctx.enter_context(tc.tile_pool(name="outs", bufs=4))
        pc = ctx.enter_context(tc.tile_pool(name="psc", bufs=2, space="PSUM"))
        pl = ctx.enter_context(tc.tile_pool(name="psl", bufs=4, space="PSUM"))

        xp_sb = consts.tile([KP, NC], f16)
        bi_sb = consts.tile([KP, 16 * 128], f16)
        # first super's audio chunk + conv weights first, so conv starts early
        c00, nc00 = SUPERS[0]
        nc.sync.dma_start(out=xp_sb[:, c00 : c00 + nc00], in_=xp_d[:, c00 : c00 + nc00])
        nc.sync.dma_start(out=bi_sb[:, :], in_=bi_d[:, :])
        cb_sb = consts.tile([128, 1], f32)
        nc.sync.dma_start(out=cb_sb[:, :], in_=cb_d[:, :])
        for c0s, ncs in SUPERS[1:]:
            nc.sync.dma_start(out=xp_sb[:, c0s : c0s + ncs], in_=xp_d[:, c0s : c0s + ncs])
        w2_sb = consts.tile([128, P], f16)
        nc.sync.dma_start(out=w2_sb[:, :], in_=w2_d[:, :])
        b2_sb = consts.tile([128, 8 * P], f32)
        nc.sync.dma_start(out=b2_sb[:, :], in_=b2_d[:, :])

        def emit_conv_step(c0, ncols, j, featss):
            """Conv MMs for I = 2j, 2j+1 into one 2-bank psum tile + ACT."""
            psc = pc.tile([128, 1024], f32)
            for u in range(2):
                I = 2 * j + u
                nc.tensor.matmul(
                    out=psc[:, u * 512 : u * 512 + ncols],
                    lhsT=bi_sb[:, I * 128 : (I + 1) * 128],
                    rhs=xp_sb[:, c0 : c0 + ncols],
                    start=True,
                    stop=True,
                )
            feats = fpool.tile([128, 1024], f16)
            if ncols == 512:
                nc.scalar.activation(
                    out=feats[:, :], in_=psc[:, :], func=RELU,
                    bias=cb_sb[:, 0:1], scale=1.0,
                )
            else:
                for u in range(2):
                    nc.scalar.activation(
                        out=feats[:, u * 512 : u * 512 + ncols],
                        in_=psc[:, u * 512 : u * 512 + ncols],
                        func=RELU, bias=cb_sb[:, 0:1], scale=1.0,
                    )
            featss.append(feats)

        def emit_linear_half(prev, cb_i, h, outt):
            """16 linear MMs (positions-in-col g in {8h..8h+7} u {16+8h..+7}) ->
            two 1-bank psum tiles (A: feats rows 0-63 only, B: rows 64-127
            only -- same-bank MMs share a PE row-group so they serialize;
            A/B pairs overlap across banks), then DVE bias-add into outt."""
            featss, c0, ncols, mlist, _outts = prev
            m0, ml = mlist[cb_i]
            pslA = pl.tile([128, 512], f32, tag="psl")
            pslB = pl.tile([128, 512], f32, tag="psl")
            for gg in range(8):
                for gpar, psl in ((0, pslA), (1, pslB)):
                    I = 8 * h + gg
                    j, u = I // 2, I % 2
                    nc.tensor.matmul(
                        out=psl[0:ml, gg * 64 : (gg + 1) * 64],
                        lhsT=featss[j][
                            gpar * 64 : (gpar + 1) * 64,
                            u * 512 + m0 : u * 512 + m0 + ml,
                        ],
                        rhs=w2_sb[gpar * 64 : (gpar + 1) * 64, :],
                        start=True,
                        stop=True,
                    )
            nc.vector.tensor_add(
                outt[0:ml, h * 512 : (h + 1) * 512],
                pslA[0:ml, 0:512],
                b2_sb[0:ml, 0:512],
            )
            nc.vector.tensor_add(
                outt[0:ml, 1024 + h * 512 : 1024 + (h + 1) * 512],
                pslB[0:ml, 0:512],
                b2_sb[0:ml, 0:512],
            )

        def emit_linear_store(prev, cb_i, outt):
            featss, c0, ncols, mlist, _outts = prev
            m0, ml = mlist[cb_i]
            pos0 = 32 * (c0 + m0)
            dview = out_d[pos0 : pos0 + 32 * ml, :].rearrange(
                "(r g) p -> r (g p)", g=32
            )
            nc.sync.dma_start(out=dview, in_=outt[0:ml, 0 : 32 * P])

        prev = None
        for c0, ncols in SUPERS:
            featss = []
            for j in range(8):
                emit_conv_step(c0, ncols, j, featss)
                if prev is not None and (j // 2) < len(prev[3]):
                    cb_i, u2 = j // 2, j % 2
                    if u2 == 0:
                        prev_outt = opool.tile([128, 2048], f16, tag="outt")
                        prev[4].append(prev_outt)
                    emit_linear_half(prev, cb_i, u2, prev[4][-1])
                    if u2 == 1:
                        emit_linear_store(prev, cb_i, prev[4][-1])
            prev = (featss, c0, ncols, col_blocks(ncols), [])

        # drain: linear for the last super
        featss, c0, ncols, mlist, outts = prev
        for cb_i in range(len(mlist)):
            outt = opool.tile([128, 2048], f16, tag="outt")
            for u2 in range(2):
                emit_linear_half(prev, cb_i, u2, outt)
            emit_linear_store(prev, cb_i, outt)


def prep_shared(conv_w, conv_b, lin_w, lin_b):
    conv_w = np.asarray(conv_w, dtype=np.float32)
    conv_b = np.asarray(conv_b, dtype=np.float32)
    lin_w = np.asarray(lin_w, dtype=np.float32)
    lin_b = np.asarray(lin_b, dtype=np.float32)

    wk = conv_w[:, 0, :]  # [64, 5]
    bi = np.zeros((KP, 16 * 128), dtype=np.float16)
    for I in range(16):
        for gpar in range(2):
            i = I + 16 * gpar
            for k in range(5):
                p = 2 * i + k
                bi[p, I * 128 + gpar * 64 : I * 128 + (gpar + 1) * 64] = wk[
                    :, k
                ].astype(np.float16)
    w2 = lin_w.T.astype(np.float16)  # [64e, 64p]
    w2s = np.ascontiguousarray(np.concatenate([w2, w2], axis=0))  # [128, 64]
    cb = np.ascontiguousarray(
        np.concatenate([conv_b, conv_b]).astype(np.float32)[:, None]
    )  # [128, 1]
    b2 = np.ascontiguousarray(
        np.tile(lin_b.astype(np.float32)[None, :], (128, 8))
    )  # [128, 1024]
    return bi, w2s, cb, b2


def prep_inputs(audio_waveform, conv_w, conv_b, lin_w, lin_b):
    x = np.asarray(audio_waveform, dtype=np.float32)
    assert x.shape == (B, T)
    bi, w2s, cb, b2 = prep_shared(conv_w, conv_b, lin_w, lin_b)

    in_maps = []
    for c in range(N_CORES):
        b_i, h = divmod(c, 2)
        P0 = h * S_CORE
        xpad = np.zeros(2 * T + 2 * 64 + 8, dtype=np.float16)
        xpad[2 : 2 + T] = x[b_i].astype(np.float16)
        sw = np.lib.stride_tricks.sliding_window_view(xpad, KP)
        xcore = np.ascontiguousarray(
            sw[2 * P0 : 2 * P0 + 64 * NC : 64].T.astype(np.float16)
        )  # [68, 3750]
        in_maps.append(dict(xp=xcore, bi=bi, w2=w2s, cb=cb, b2=b2))
    return in_maps


_NC_CACHE = None


def get_nc() -> bass.Bass:
    global _NC_CACHE
    if _NC_CACHE is None:
        nc = bacc.Bacc()
        emit(nc)
        nc.compile()
        _NC_CACHE = nc
    return _NC_CACHE


def run(inputs: dict, trace: bool = False):
    in_maps = prep_inputs(**inputs)
    nc = get_nc()
    res = run_bass_kernel_spmd(nc, in_maps, list(range(N_CORES)), trace=trace)
    out = np.empty((B, S_FULL, P), dtype=np.float32)
    for c in range(N_CORES):
        b_i, h = divmod(c, 2)
        out[b_i, h * S_CORE : (h + 1) * S_CORE, :] = res.results[c]["out"].astype(
            np.float32
        )
    return out, res


def kernel(**inputs) -> np.ndarray:
    out, _ = run(inputs)
    return out


# revision 8
# speedup vs baseline: 1.8477x; 1.1699x over previous
"""AudioEncoder Trainium2 kernel, v2 (column-major conv scheme).

Computes: conv1d(1->64, k=5, stride=2, pad=2) + bias -> ReLU -> per-timestep
linear (64->64) + bias, over audio [4, 480000] f32 -> out [4, 240000, 64] f32.

Layout strategy (8 cores, pure data parallel; core = half of one batch row,
S = 120000 output positions):

  Host builds xcore [68, 3750] fp16 per core, xcore[p, c] = xpad[64c + p]
  (xpad = zero-padded audio, stride-2 conv => 64 input samples per 32 output
  positions).  Column c covers output positions 32c..32c+31; position
  i within a column needs taps xpad[64c + 2i .. 2i+4].

  Conv as matmul with 16 constant stationaries B_I [68, 128] fp16:
  B_I[p, gpar*64+ch] = wk[ch, p - 2*(I + 16*gpar)].  MM_I: psum[128, ncols]
  = B_I.T @ xcore[:, c0:c0+ncols] gives channels of positions (I, I+16) of
  every column -- plain contiguous moving operand, position permutation is
  free.  ACT applies conv bias + ReLU -> feats_j fp16 (j = I//2, two I per
  [128,1024] psum tile).

  Linear: per col-block of ml<=128 columns, 32 matmuls (g = I + 16*gpar):
  lhsT = feats_j[gpar*64:+64, u*512+m0:+ml] (stationary), rhs = w2 dup
  [gpar*64:+64, 0:64] -> psl[r, g*64+p] = out(pos 32*(c0+m0+r)+g, feat p).
  The gpar=0 / gpar=1 halves write DIFFERENT psum banks: the PE overlaps
  row-group-disjoint matmuls, and two concurrent matmuls writing the same
  partitions of one bank is a hardware fault (write-port conflict).  Within
  a bank all matmuls share one row-group, so they serialize safely.
  DVE adds (pre-broadcast) lin bias, casting f32 psum -> fp16 out tile.

  Store: outt[r, 0:2048] is exactly out[pos0+32r .. pos0+32r+31, 0:64] --
  one fully contiguous 4 KiB descriptor per partition, 512 KiB per store.
  Output is fp16 on device (quantization ~2.4e-4 << 2e-2 tol); host casts
  back to f32.

  PSUM: conv pool 2 bufs x 2 banks + linear pool 2 bufs x 2 banks = 8 banks.
"""

import numpy as np

import concourse.bacc as bacc
import concourse.bass as bass
import concourse.mybir as mybir
import concourse.tile as tile
from concourse.bass_utils import run_bass_kernel_spmd

B = 4
T = 480000
S_FULL = 240000
N_CORES = 8
S_CORE = S_FULL * B // N_CORES  # 120000
NC = S_CORE // 32  # 3750 columns per core
KP = 128  # xcore partitions (taps use rows 0-66; padded to 128 for DMA port
# coverage on loads and full-row stationaries)
E = 64
P = 64

f16 = mybir.dt.float16
f32 = mybir.dt.float32

SUPERS = [(j * 512, 512) for j in range(NC // 512)] + (
    [((NC // 512) * 512, NC % 512)] if NC % 512 else []
)


def col_blocks(ncols):
    out = []
    m0 = 0
    while m0 < ncols:
        out.append((m0, min(128, ncols - m0)))
        m0 += 128
    return out


def emit(nc: bass.Bass) -> None:
    from contextlib import ExitStack

    xp_d = nc.declare_dram_parameter("xp", [KP, NC], f16, isOutput=False)
    bi_d = nc.declare_dram_parameter("bi", [KP, 16 * 128], f16, isOutput=False)
    w2_d = nc.declare_dram_parameter("w2", [128, P], f16, isOutput=False)
    cb_d = nc.declare_dram_parameter("cb", [128, 1], f32, isOutput=False)
    b2_d = nc.declare_dram_parameter("b2", [128, 8 * P], f32, isOutput=False)
    out_d = nc.declare_dram_parameter("out", [S_CORE, P], f16, isOutput=True)

    RELU = mybir.ActivationFunctionType.Relu

    with tile.TileContext(nc) as tc, ExitStack() as ctx:
        consts = ctx.enter_context(tc.tile_pool(name="consts", bufs=1))
        fpool = ctx.enter_context(tc.tile_pool(name="feats", bufs=24))
        opool = ctx.enter_context(tc.tile_pool(name="outs", bufs=6))
        pc = ctx.enter_context(tc.tile_pool(name="psc", bufs=2, space="PSUM"))
        pl = ctx.enter_context(tc.tile_pool(name="psl", bufs=4, space="PSUM"))

        xp_sb = consts.tile([KP, NC], f16)
        bi_sb = consts.tile([KP, 16 * 128], f16)
        # first super's audio chunk + first conv weights first, so conv starts
        # early; the rest of bi and the remaining audio chunks follow
        c00, nc00 = SUPERS[0]
        nc.sync.dma_start(out=bi_sb[:, 0 : 4 * 128], in_=bi_d[:, 0 : 4 * 128])
        nc.sync.dma_start(out=xp_sb[:, c00 : c00 + nc00], in_=xp_d[:, c00 : c00 + nc00])
        cb_sb = consts.tile([128, 1], f32)
        nc.sync.dma_start(out=cb_sb[:, :], in_=cb_d[:, :])
        nc.sync.dma_start(out=bi_sb[:, 4 * 128 :], in_=bi_d[:, 4 * 128 :])
        for c0s, ncs in SUPERS[1:]:
            nc.sync.dma_start(out=xp_sb[:, c0s : c0s + ncs], in_=xp_d[:, c0s : c0s + ncs])
        w2_sb = consts.tile([128, P], f16)
        nc.sync.dma_start(out=w2_sb[:, :], in_=w2_d[:, :])
        b2_sb = consts.tile([128, 8 * P], f32)
        nc.sync.dma_start(out=b2_sb[:, :], in_=b2_d[:, :])

        def emit_conv_step(c0, ncols, j, featss):
            """Conv MMs for I = 2j, 2j+1 into one 2-bank psum tile + ACT."""
            psc = pc.tile([128, 1024], f32)
            for u in range(2):
                I = 2 * j + u
                nc.tensor.matmul(
                    out=psc[:, u * 512 : u * 512 + ncols],
                    lhsT=bi_sb[:, I * 128 : (I + 1) * 128],
                    rhs=xp_sb[:, c0 : c0 + ncols],
                    start=True,
                    stop=True,
                )
            feats = fpool.tile([128, 1024], f16)
            if ncols == 512:
                nc.scalar.activation(
                    out=feats[:, :], in_=psc[:, :], func=RELU,
                    bias=cb_sb[:, 0:1], scale=1.0,
                )
            else:
                for u in range(2):
                    nc.scalar.activation(
                        out=feats[:, u * 512 : u * 512 + ncols],
                        in_=psc[:, u * 512 : u * 512 + ncols],
                        func=RELU, bias=cb_sb[:, 0:1], scale=1.0,
                    )
            featss.append(feats)

        def emit_linear_half(prev, cb_i, h, outt):
            """16 linear MMs (positions-in-col g in {8h..8h+7} u {16+8h..+7}) ->
            two 1-bank psum tiles (A: feats rows 0-63 only, B: rows 64-127
            only -- same-bank MMs share a PE row-group so they serialize;
            A/B pairs overlap across banks), then DVE bias-add into outt."""
            featss, c0, ncols, mlist, _outts = prev
            m0, ml = mlist[cb_i]
            pslA = pl.tile([128, 512], f32, tag="psl")
            pslB = pl.tile([128, 512], f32, tag="psl")
            for gg in range(8):
                for gpar, psl in ((0, pslA), (1, pslB)):
                    I = 8 * h + gg
                    j, u = I // 2, I % 2
                    nc.tensor.matmul(
                        out=psl[0:ml, gg * 64 : (gg + 1) * 64],
                        lhsT=featss[j][
                            gpar * 64 : (gpar + 1) * 64,
                            u * 512 + m0 : u * 512 + m0 + ml,
                        ],
                        rhs=w2_sb[gpar * 64 : (gpar + 1) * 64, :],
                        start=True,
                        stop=True,
                    )
            nc.vector.tensor_add(
                outt[0:ml, h * 512 : (h + 1) * 512],
                pslA[0:ml, 0:512],
                b2_sb[0:ml, 0:512],
            )
            nc.vector.tensor_add(
                outt[0:ml, 1024 + h * 512 : 1024 + (h + 1) * 512],
                pslB[0:ml, 0:512],
                b2_sb[0:ml, 0:512],
            )

        def emit_linear_store(prev, cb_i, outt):
            featss, c0, ncols, mlist, _outts = prev
            m0, ml = mlist[cb_i]
            pos0 = 32 * (c0 + m0)
            dview = out_d[pos0 : pos0 + 32 * ml, :].rearrange(
                "(r g) p -> r (g p)", g=32
            )
            nc.sync.dma_start(out=dview, in_=outt[0:ml, 0 : 32 * P])

        prev = None
        for c0, ncols in SUPERS:
            featss = []
            for j in range(8):
                emit_conv_step(c0, ncols, j, featss)
                if prev is not None and (j // 2) < len(prev[3]):
                    cb_i, u2 = j // 2, j % 2
                    if u2 == 0:
                        prev_outt = opool.tile([128, 2048], f16, tag="outt")
                        prev[4].append(prev_outt)
                    emit_linear_half(prev, cb_i, u2, prev[4][-1])
                    if u2 == 1:
                        emit_linear_store(prev, cb_i, prev[4][-1])
            prev = (featss, c0, ncols, col_blocks(ncols), [])

        # drain: linear for the last super
        featss, c0, ncols, mlist, outts = prev
        for cb_i in range(len(mlist)):
            outt = opool.tile([128, 2048], f16, tag="outt")
            for u2 in range(2):
                emit_linear_half(prev, cb_i, u2, outt)
            emit_linear_store(prev, cb_i, outt)


def prep_shared(conv_w, conv_b, lin_w, lin_b):
    conv_w = np.asarray(conv_w, dtype=np.float32)
    conv_b = np.asarray(conv_b, dtype=np.float32)
    lin_w = np.asarray(lin_w, dtype=np.float32)
    lin_b = np.asarray(lin_b, dtype=np.float32)

    wk = conv_w[:, 0, :]  # [64, 5]
    bi = np.zeros((KP, 16 * 128), dtype=np.float16)
    for I in range(16):
        for gpar in range(2):
            i = I + 16 * gpar
            for k in range(5):
                p = 2 * i + k
                bi[p, I * 128 + gpar * 64 : I * 128 + (gpar + 1) * 64] = wk[
                    :, k
                ].astype(np.float16)
    w2 = lin_w.T.astype(np.float16)  # [64e, 64p]
    w2s = np.ascontiguousarray(np.concatenate([w2, w2], axis=0))  # [128, 64]
    cb = np.ascontiguousarray(
        np.concatenate([conv_b, conv_b]).astype(np.float32)[:, None]
    )  # [128, 1]
    b2 = np.ascontiguousarray(
        np.tile(lin_b.astype(np.float32)[None, :], (128, 8))
    )  # [128, 1024]
    return bi, w2s, cb, b2


def prep_inputs(audio_waveform, conv_w, conv_b, lin_w, lin_b):
    x = np.asarray(audio_waveform, dtype=np.float32)
    assert x.shape == (B, T)
    bi, w2s, cb, b2 = prep_shared(conv_w, conv_b, lin_w, lin_b)

    in_maps = []
    for c in range(N_CORES):
        b_i, h = divmod(c, 2)
        P0 = h * S_CORE
        xpad = np.zeros(2 * T + 2 * 64 + 8, dtype=np.float16)
        xpad[2 : 2 + T] = x[b_i].astype(np.float16)
        sw = np.lib.stride_tricks.sliding_window_view(xpad, KP)
        xcore = np.ascontiguousarray(
            sw[2 * P0 : 2 * P0 + 64 * NC : 64].T.astype(np.float16)
        )  # [68, 3750]
        in_maps.append(dict(xp=xcore, bi=bi, w2=w2s, cb=cb, b2=b2))
    return in_maps


_NC_CACHE = None


def get_nc() -> bass.Bass:
    global _NC_CACHE
    if _NC_CACHE is None:
        nc = bacc.Bacc()
        emit(nc)
        nc.compile()
        _NC_CACHE = nc
    return _NC_CACHE


def run(inputs: dict, trace: bool = False):
    in_maps = prep_inputs(**inputs)
    nc = get_nc()
    res = run_bass_kernel_spmd(nc, in_maps, list(range(N_CORES)), trace=trace)
    out = np.empty((B, S_FULL, P), dtype=np.float32)
    for c in range(N_CORES):
        b_i, h = divmod(c, 2)
        out[b_i, h * S_CORE : (h + 1) * S_CORE, :] = res.results[c]["out"].astype(
            np.float32
        )
    return out, res


def kernel(**inputs) -> np.ndarray:
    out, _ = run(inputs)
    return out
